# revision 26
# baseline (speedup 1.0000x reference)
"""LSTM encoder (embedding gather + 512-step LSTM) on 8 TRN2 NeuronCores.

Sharding: data-parallel over batch — each of the 8 cores owns 8 of the 64
sequences end-to-end (embedding table and weights replicated), so the
sequential recurrence needs no cross-core communication.

Per-core Bass/Tile kernel (v2, software-pipelined):
  The hardware loop body covers 32 LSTM steps (two 16-step m-tiles A/B) so
  the x-phase (embedding gather -> X.T -> xg = X @ W_ih) can double-buffer
  entirely in SBUF, one m-tile ahead of the recurrence, with its 64 matmuls
  spread 4-per-step to keep the PE's HAM clock-gate warm through each step's
  ACT/DVE tail.

  Per step: bank0 (gate cols 0:512) accumulates h.T @ W_hh over all 8 k-tiles
  first, so its PSUM->SBUF evacuation, PE transposes and sigmoid/tanh chain
  overlap the bank1 matmuls; bias rides extra rows of the xg seed scatter.
  The half-H cell chains produce h.T strips 0..3 early so the next step's
  k=0..3 matmuls start while this step's second half-chain is still running.
All matmuls are bf16 with fp32 PSUM accumulation; the cell state is fp32.
"""
import sys

if "/opt/trn_rl_repo" not in sys.path:
    sys.path.insert(0, "/opt/trn_rl_repo")

import numpy as np
import ml_dtypes
import concourse.bass as bass
import concourse.tile as tile
from concourse import bacc, mybir
from concourse.masks import make_identity

F32 = mybir.dt.float32
BF16 = mybir.dt.bfloat16
I32 = mybir.dt.int32
P = 128
GATE_PERM = [0, 1, 3, 2]  # strip j -> original gate block (W order: i, f, g, o)

# Problem constants (hardcoded per contest contract)
VOCAB, E, H = 32000, 1024, 1024
B, S = 64, 512
NCORES = 8
BLOC = B // NCORES
U = 16

_program_cache = {}


def build_program(S=S, BLOC=BLOC, E=E, H=H, VOCAB=VOCAB, U=U):
    KT = E // P
    KTH = H // P
    GN = 4 * H
    TOK = S * BLOC
    NIT = S // U          # m-tiles
    NB = NIT // 2         # hardware-loop bodies (2 m-tiles each)
    JB = 4 * BLOC
    assert U == 16 and S % (2 * U) == 0 and TOK // P == NIT

    nc = bacc.Bacc(None, target_bir_lowering=False, debug=False)

    src_idx = nc.dram_tensor("src_idx", [TOK + 2 * P, 1], I32, kind="ExternalInput")
    emb = nc.dram_tensor("emb", [VOCAB, E], BF16, kind="ExternalInput")
    wih = nc.dram_tensor("wih", [P, KT, GN], BF16, kind="ExternalInput")
    whh = nc.dram_tensor("whh", [P, KTH, GN], BF16, kind="ExternalInput")
    bias_rows = nc.dram_tensor("bias_rows", [4, U * H], BF16, kind="ExternalInput")
    scat = nc.dram_tensor("scat", [JB + 4, P], BF16, kind="ExternalInput")
    hs = nc.dram_tensor("hs", [S, P, BLOC * KTH], BF16, kind="ExternalOutput")
    xg_hbm = nc.dram_tensor("xg_hbm", [S + 2 * U, JB, H], BF16)

    with tile.TileContext(nc) as tc:
        with tc.tile_pool(name="const", bufs=1) as const, \
             tc.tile_pool(name="rw", bufs=1) as rw, \
             tc.tile_pool(name="state", bufs=1) as state, \
             tc.tile_pool(name="rsb", bufs=1) as rsb, \
             tc.tile_pool(name="rps", bufs=2, space="PSUM") as rps, \
             tc.tile_pool(name="gtps_pool", bufs=1, space="PSUM") as gtps_pool, \
             tc.tile_pool(name="xtp", bufs=1, space="PSUM") as xtp, \
             tc.tile_pool(name="xgp", bufs=2, space="PSUM") as xgp:
            ident = const.tile([P, P], BF16)
            make_identity(nc, ident[:])
            whh_sb = rw.tile([P, KTH, GN], BF16)
            nc.sync.dma_start(out=whh_sb[:], in_=whh[:])
            wih_sb = rw.tile([P, KT, GN], BF16)
            nc.sync.dma_start(out=wih_sb[:], in_=wih[:])
            scat_sb = rw.tile([JB + 4, P], BF16)
            nc.sync.dma_start(out=scat_sb[:], in_=scat[:])

            hT = [state.tile([P, KTH * 32], BF16, tag=f"hT{i}", name=f"hT{i}")
                  for i in range(2)]
            cst = [state.tile([P, BLOC * KTH], F32, tag=f"cst{i}", name=f"cst{i}")
                   for i in range(2)]
            nc.vector.memset(hT[0][:], 0.0)
            nc.vector.memset(hT[1][:], 0.0)
            nc.vector.memset(cst[0][:], 0.0)
            # double-buffered per-m-tile state (parity = m-tile index % 2)
            xg_it = [state.tile([JB + 4, U * H], BF16, tag=f"xgit{i}", name=f"xgit{i}")
                     for i in range(2)]
            for i in range(2):
                nc.sync.dma_start(out=xg_it[i][JB:JB + 4, :], in_=bias_rows[:])
            hs_it = [state.tile([P, (U // 2) * BLOC * KTH], BF16, tag=f"hsit{i}",
                                name=f"hsit{i}") for i in range(2)]
            xt_sb = [state.tile([P, KT * P], BF16, tag=f"xt{i}", name=f"xt{i}") for i in range(2)]
            idx_sb = state.tile([P, 1], I32, tag="idx")
            xrow = state.tile([P, E], BF16, tag="xrow")

            # ---------------- x-phase pieces ----------------
            def x_prep(us, mt, par):
                """Slice us (0..15) of gather/transpose prep for m-tile mt
                (int or scalar expr) into xt_sb[par]."""
                if us == 0:
                    nc.sync.dma_start(out=idx_sb[:],
                                      in_=src_idx[bass.ds(mt * P, P), :])
                    nc.gpsimd.indirect_dma_start(
                        out=xrow[:], out_offset=None, in_=emb[:],
                        in_offset=bass.IndirectOffsetOnAxis(ap=idx_sb[:, :1], axis=0))
                elif 2 <= us <= 5:
                    for c in (2 * (us - 2), 2 * (us - 2) + 1):
                        xt_ps = xtp.tile([P, P], BF16, tag="xtps")
                        nc.tensor.transpose(out=xt_ps[:],
                                            in_=xrow[:, c * P:(c + 1) * P],
                                            identity=ident[:])
                        nc.scalar.copy(out=xt_sb[par][:, c * P:(c + 1) * P],
                                       in_=xt_ps[:])

            xg_acc = {}    # jn -> psum tile accumulating xg
            xg_done = []   # (psum tile, jn, mt, par) ready for evacuation
            xg_loadq = []  # (jn, mt, par) written to HBM, ready to prefetch

            def x_mm(us, mt, par_src, par_dst):
                """xg matmuls for slot us: slots 0/1 do full slices jn=0/1,
                slots 2..13 do half-slices of jn=2..7, so the last slice's
                HBM round trip completes within the 16-slot budget."""
                if us >= 14:
                    return
                if us <= 1:
                    jn, kr = us, range(KT)
                else:
                    jn, kr = us // 2 + 1, range(4 * (us % 2), 4 * (us % 2) + 4)
                if kr.start == 0:
                    xg_acc[jn] = xgp.tile([P, 512], F32, tag="xgps", name="xgps")
                xg_ps = xg_acc[jn]
                for k in kr:
                    nc.tensor.matmul(
                        out=xg_ps[:], lhsT=xt_sb[par_src][:, k * P:(k + 1) * P],
                        rhs=wih_sb[:, k, jn * 512:(jn + 1) * 512],
                        start=(k == 0), stop=(k == KT - 1))
                if kr.stop == KT:
                    xg_done.append((xg_acc.pop(jn), jn, mt, par_dst))

            def x_out():
                """Evacuate one finished xg slice: cast fp32->bf16 on DVE,
                DMA to xg_hbm (the DMA performs token-partition -> step-row)."""
                if not xg_done:
                    return
                xg_ps, jn, mt, par = xg_done.pop(0)
                j, nh = jn // 2, jn % 2
                xgq = rsb.tile([P, 512], BF16, tag="xgq")
                nc.vector.tensor_copy(out=xgq[:], in_=xg_ps[:])
                nc.sync.dma_start(
                    out=xg_hbm[bass.ds(mt * U, U), j * BLOC:(j + 1) * BLOC,
                               nh * 512:(nh + 1) * 512],
                    in_=xgq[:])
                xg_loadq.append((jn, mt, par))

            def x_load():
                """Prefetch one evacuated slice from xg_hbm into xg_it[par]."""
                if not xg_loadq:
                    return
                jn, mt, par = xg_loadq.pop(0)
                j, nh = jn // 2, jn % 2
                slt = xg_it[par][j * BLOC:(j + 1) * BLOC, :]
                dst = bass.AP(tensor=slt.tensor, offset=slt.offset + nh * 512,
                              ap=[slt.ap[0], [H, U], [1, 512]])
                nc.sync.dma_start(
                    out=dst,
                    in_=xg_hbm[bass.ds(mt * U, U), j * BLOC:(j + 1) * BLOC,
                               nh * 512:(nh + 1) * 512].rearrange("t p h -> p t h"))

            # ---------------- recurrence ----------------
            pend = {}

            def seeds(g):
                """Allocate step g's gate PSUM and seed xg+bias via scatter."""
                u, par = g % U, (g // U) % 2
                g_ps = rps.tile([P, 1024], F32, tag="gps")
                for n in range(2):
                    nc.tensor.matmul(
                        out=g_ps[:, 512 * n:512 * (n + 1)],
                        lhsT=scat_sb[:, :],
                        rhs=xg_it[par][:, u * H + 512 * n: u * H + 512 * (n + 1)],
                        start=True, stop=True)
                pend[g] = g_ps

            def hh(g_ps, h_cur, n, ks, start_k=None):
                for k in ks:
                    for j in range(4):
                        nc.tensor.matmul(
                            out=g_ps[32 * j:32 * (j + 1), 512 * n:512 * (n + 1)],
                            lhsT=h_cur[:, 32 * k:32 * (k + 1)],
                            rhs=whh_sb[:, k, j * H + 512 * n: j * H + 512 * (n + 1)],
                            start=False, stop=(k == KTH - 1),
                            tile_position=(0, 32 * j),
                            skip_group_check=True)

            def chain(nh, g_ps, gt_ps, c_cur, c_new, hs_slot, h_new, u):
                """Half-H activation + cell update for chunks c in
                [4*nh, 4*nh+4); writes h strips 4*nh..4*nh+4."""
                base = gt_ps[:]

                def gt_src(j0, nj):
                    return bass.AP(tensor=base.tensor,
                                   offset=base.offset + 32 * j0 + 512 * nh,
                                   ap=[base.ap[0], [32, nj], [P, KTH // 2],
                                       [1, BLOC]])

                cs = slice(32 * nh, 32 * nh + 32)
                s_ifo = rsb.tile([P, 96], F32, tag=f"sifo{nh}")
                nc.scalar.activation(
                    out=s_ifo[:].rearrange("p (j c b) -> p j c b", j=3, c=KTH // 2),
                    in_=gt_src(0, 3),
                    func=mybir.ActivationFunctionType.Sigmoid)
                t_g = rsb.tile([P, 32], F32, tag=f"tg{nh}")
                nc.scalar.activation(
                    out=t_g[:].rearrange("p (j c b) -> p j c b", j=1, c=KTH // 2),
                    in_=gt_src(3, 1),
                    func=mybir.ActivationFunctionType.Tanh)
                fc = rsb.tile([P, 32], F32, tag=f"fc{nh}")
                nc.vector.tensor_tensor(out=fc[:], in0=c_cur[:, cs],
                                        in1=s_ifo[:, 32:64],
                                        op=mybir.AluOpType.mult)
                ig = rsb.tile([P, 32], F32, tag=f"ig{nh}")
                nc.vector.tensor_tensor(out=ig[:], in0=t_g[:], in1=s_ifo[:, 0:32],
                                        op=mybir.AluOpType.mult)
                nc.vector.tensor_tensor(out=c_new[:, cs], in0=fc[:], in1=ig[:],
                                        op=mybir.AluOpType.add)
                t_c = rsb.tile([P, 32], F32, tag=f"tc{nh}")
                nc.scalar.activation(out=t_c[:], in_=c_new[:, cs],
                                     func=mybir.ActivationFunctionType.Tanh)
                hout = hs_slot[:, (u % 8) * 64 + 32 * nh: (u % 8) * 64 + 32 * nh + 32]
                nc.vector.tensor_tensor(out=hout, in0=t_c[:], in1=s_ifo[:, 64:96],
                                        op=mybir.AluOpType.mult)
                hT_dst = bass.AP(
                    tensor=h_new.tensor,
                    offset=h_new[:].offset + 32 * (KTH // 2) * nh,
                    ap=[h_new[:].ap[0], [32, KTH // 2], [1, BLOC]])
                nc.vector.tensor_copy(
                    out=hT_dst,
                    in_=hout.rearrange("p (c b) -> p c b", c=KTH // 2))

            def step(g, fill):
                """One LSTM step; g in [0, 2U) is the body-local step index.
                fill() emits this slot's x-phase work (PE part lands between
                tr1 and the next step's matmuls)."""
                u, par = g % U, (g // U) % 2
                h_cur, h_new = hT[g % 2], hT[(g + 1) % 2]
                c_cur, c_new = cst[g % 2], cst[(g + 1) % 2]
                if g not in pend:
                    seeds(g)
                g_ps = pend.pop(g)
                g_sb = rsb.tile([P, 1024], BF16, tag="gsb")
                gt_ps = gtps_pool.tile([P, 1024], BF16, tag="gtps")
                hs_slot = hs_it[u // 8]

                hh(g_ps, h_cur, 0, range(KTH))           # bank0, all k
                nc.scalar.copy(out=g_sb[:, 0:512], in_=g_ps[:, 0:512])
                hh(g_ps, h_cur, 1, range(KTH // 2))      # bank1 k0..3
                for c in range(KTH // 2):                # tr0
                    nc.tensor.transpose(out=gt_ps[:, c * P:(c + 1) * P],
                                        in_=g_sb[:, c * P:(c + 1) * P],
                                        identity=ident[:])
                hh(g_ps, h_cur, 1, range(KTH // 2, KTH))  # bank1 k4..7
                nc.vector.tensor_copy(out=g_sb[:, 512:1024], in_=g_ps[:, 512:1024])
                for c in range(KTH // 2, KTH):           # tr1
                    nc.tensor.transpose(out=gt_ps[:, c * P:(c + 1) * P],
                                        in_=g_sb[:, c * P:(c + 1) * P],
                                        identity=ident[:])
                chain(0, g_ps, gt_ps, c_cur, c_new, hs_slot, h_new, u)
                fill()
                if g + 1 < 2 * U:
                    seeds(g + 1)
                chain(1, g_ps, gt_ps, c_cur, c_new, hs_slot, h_new, u)

            def body_half(half, mt_cur, mt_next, mt_prep):
                """16 steps for m-tile mt_cur; x-phase computes xg(mt_next)
                from xt_sb[par of mt_next] and preps xt for mt_prep."""
                par = half  # parity of mt_cur within the body
                for u in range(U):
                    def fill(u=u):
                        x_out()
                        x_load()
                        x_prep(u, mt_prep, par)
                        x_mm(u, mt_next, 1 - par, 1 - par)
                    step(half * U + u, fill)
                    if u % 8 == 7:
                        ch = u // 8
                        nc.sync.dma_start(
                            out=hs[bass.ds(mt_cur * U + ch * 8, 8), :, :]
                            .rearrange("t p c -> p t c"),
                            in_=hs_it[ch][:].rearrange("p (t c) -> p t c", t=8))

            # ---------------- prologue ----------------
            # xg(m-tile 0) -> xg_it[0]; xt(m-tile 1) -> xt_sb[1]
            for us in range(6):
                x_prep(us, 0, 0)
            for us in range(U):
                x_mm(us, 0, 0, 0)
                x_out()
                x_load()
            x_out()
            x_load()
            x_load()
            for us in range(6):
                x_prep(us, 1, 1)

            with tc.For_i(0, NB, 1) as iv:
                # half 0: recur m-tile 2iv   (xg in buf0), compute xg(2iv+1)->buf1, prep xt(2iv+2)->buf0
                body_half(0, iv * 2, iv * 2 + 1, iv * 2 + 2)
                # half 1: recur m-tile 2iv+1 (xg in buf1), compute xg(2iv+2)->buf0, prep xt(2iv+3)->buf1
                body_half(1, iv * 2 + 1, iv * 2 + 2, iv * 2 + 3)
                pend.clear()

    nc.compile()
    return nc


def _prep_inputs(source, embedding, W_ih, W_hh, b, core, n_cores=NCORES):
    src_k = np.asarray(source[core * BLOC:(core + 1) * BLOC, :], dtype=np.int32)
    idx = np.ascontiguousarray(src_k.T.reshape(-1, 1))  # (t-major, b)
    idx = np.concatenate([idx, np.zeros((2 * P, 1), np.int32)], axis=0)  # slack

    def prep_w(W, K):
        Wr = np.asarray(W, np.float32).reshape(K // P, P, 4, H)[:, :, GATE_PERM, :]
        return np.ascontiguousarray(
            Wr.transpose(1, 0, 2, 3).reshape(P, K // P, 4 * H)).astype(ml_dtypes.bfloat16)

    bias_dev = np.ascontiguousarray(
        np.asarray(b, np.float32).reshape(4, H)[GATE_PERM].reshape(4 * H))
    bias_rows = np.tile(bias_dev.reshape(4, H), (1, U))
    JB = 4 * BLOC
    scat = np.zeros((JB + 4, P), np.float32)
    for j in range(4):
        for bb in range(BLOC):
            scat[j * BLOC + bb, 32 * j + bb] = 1.0
            scat[JB + j, 32 * j + bb] = 1.0  # bias row feeds gate strip j
    return {
        "src_idx": idx,
        "emb": np.asarray(embedding, np.float32).astype(ml_dtypes.bfloat16),
        "wih": prep_w(W_ih, E),
        "whh": prep_w(W_hh, H),
        "bias_rows": bias_rows.astype(ml_dtypes.bfloat16),
        "scat": scat.astype(ml_dtypes.bfloat16),
    }


def _unpack_output(hs_dev):
    KTH = H // P
    a = np.asarray(hs_dev, dtype=np.float32).reshape(S, P, KTH, BLOC)
    return np.ascontiguousarray(a.transpose(3, 0, 2, 1)).reshape(BLOC, S, H)


def _get_program():
    if "nc" not in _program_cache:
        _program_cache["nc"] = build_program()
    return _program_cache["nc"]


def kernel(source, embedding, W_ih, W_hh, b):
    """Full inputs in, full output out. Shards batch over 8 NeuronCores."""
    from concourse import bass2jax

    source = np.asarray(source)
    embedding = np.asarray(embedding, np.float32)
    W_ih = np.asarray(W_ih, np.float32)
    W_hh = np.asarray(W_hh, np.float32)
    b = np.asarray(b, np.float32)

    nc = _get_program()
    in_maps = [_prep_inputs(source, embedding, W_ih, W_hh, b, core=k)
               for k in range(NCORES)]
    res = bass2jax.run_bass_via_pjrt(nc, in_maps, n_cores=NCORES)
    out = np.concatenate([_unpack_output(res[k]["hs"]) for k in range(NCORES)],
                         axis=0)
    return out.astype(np.float32)
